# revision 18
# baseline (speedup 1.0000x reference)
"""Trainium2 Bass kernel for nn_MemTransformerLM (Transformer-XL rel-attention).

Sequence-parallel over 8 NeuronCores: core m owns token span [260m, 260(m+1)).

v2 restructure vs baseline:
- Per-head DRAM rel-shift buffers with row pitch 2T+1: each row is
  [zero][E (T)][mask (T)], where the mask region holds -1e30 so the
  Transformer-XL shift's wrap-read lands on the additive causal mask for
  free (no on-chip mask add). Mem-token corner "garbage" (which the
  reference keeps) is reproduced by patching the first 15 mask cells of the
  32 corner rows each layer with raw E values via tiny matmuls + blends
  driven by per-core blend masks (identity off the corner cores).
- BD is accumulated into the AC PSUM via an identity matmul, and softmax's
  exp reads PSUM directly with the 1/sqrt(d) scale folded into the
  activation, eliminating the separate mask-add/score ops.
- One combined K+V AllGather per layer, overlapped with the (K/V
  independent) E pass.
- Batched 3D-AP DMAs for weights, posT, K/V gather, and E/BD round trips.
"""

import numpy as np
import ml_dtypes

import concourse.bass as bass
import concourse.mybir as mybir
import concourse.tile as tile
from concourse import bacc
from concourse.bass import ds
from concourse.bass_utils import run_bass_kernel_spmd
from concourse.masks import make_identity

BF16 = ml_dtypes.bfloat16
DT = mybir.dt
AF = mybir.ActivationFunctionType
ALU = mybir.AluOpType

N_LAYER = 4
N_HEAD = 8
D_HEAD = 64
D_MODEL = 512
D_INNER = 2048
NMT = 16
T = 2048 + 2 * NMT      # 2080
N_CORES = 8
SPAN = T // N_CORES     # 260
QT_REAL = [128, 128, 4]
QT_W = [128, 128, 16]   # on-chip partition width per q-tile (>=16 for xbar)
N_QT = 3
LR = 272                # local-row extent (2*128 + 16)
NEG = -1e30
SCALE = 1.0 / np.sqrt(D_HEAD)
TPAD = 17 * 128         # 2176
RW = 2 * T + 1          # P-buffer row pitch: [zero][E(T)][M(T)]
P_ROWS = 384
CC_W = [512, 512, 512, 512, 32]  # exp/psum chunking of the j axis
N_CC = 5
HP = 4                  # head-pair partition tiles


def _host_prep(inputs):
    word_emb = np.asarray(inputs["word_emb"], np.float32)
    mem_tokens = np.asarray(inputs["mem_tokens"], np.float32)
    w = np.transpose(word_emb, (1, 0, 2))[:, 0, :]
    mem = mem_tokens[:, 0, :]
    w_full = np.concatenate([mem, w, mem], axis=0)          # [T, 512]

    inv_freq = 1.0 / (10000.0 ** (np.arange(0, D_MODEL, 2, dtype=np.float32) / D_MODEL))
    pos_seq = np.arange(T - 1, -1, -1.0, dtype=np.float32)
    sinusoid = pos_seq[:, None] * inv_freq[None, :]
    pos_emb = np.concatenate([np.sin(sinusoid), np.cos(sinusoid)], axis=-1)
    posT = np.ascontiguousarray(pos_emb.T).astype(BF16)     # [512, T]

    wqkv = np.asarray(inputs["Wqkv"], np.float32).astype(BF16)
    wr = np.asarray(inputs["Wr"], np.float32).astype(BF16)
    wo = np.asarray(inputs["Wo"], np.float32).astype(BF16)
    w1 = np.asarray(inputs["ffn_W1"], np.float32).astype(BF16)
    w2 = np.asarray(inputs["ffn_W2"], np.float32).astype(BF16)
    rwb = np.ascontiguousarray(np.asarray(inputs["r_w_bias"], np.float32).reshape(-1, 1))
    rrb = np.ascontiguousarray(np.asarray(inputs["r_r_bias"], np.float32).reshape(-1, 1))

    ln1g = np.asarray(inputs["ln1_scale"], np.float32)
    ln1b = np.asarray(inputs["ln1_bias"], np.float32)
    ln2g = np.asarray(inputs["ln2_scale"], np.float32)
    ln2b = np.asarray(inputs["ln2_bias"], np.float32)
    gb = np.ascontiguousarray(
        np.stack(
            [
                np.broadcast_to(ln1g[:, None, :], (N_LAYER, 128, D_MODEL)),
                np.broadcast_to(ln1b[:, None, :], (N_LAYER, 128, D_MODEL)),
                np.broadcast_to(ln2g[:, None, :], (N_LAYER, 128, D_MODEL)),
                np.broadcast_to(ln2b[:, None, :], (N_LAYER, 128, D_MODEL)),
            ],
            axis=2,
        ).astype(np.float32)
    )
    b1col = np.ascontiguousarray(
        np.asarray(inputs["ffn_b1"], np.float32).reshape(N_LAYER, D_INNER, 1)
    )
    b2bc = np.ascontiguousarray(
        np.broadcast_to(
            np.asarray(inputs["ffn_b2"], np.float32)[:, None, :],
            (N_LAYER, 128, D_MODEL),
        ).copy()
    )

    # Corner blend masks: tri[c, y] selects the wrap-garbage region
    # (y < 15-c) of the patched mask cells; neg fills the rest with -1e30.
    tri = np.zeros((16, 15), np.float32)
    for c in range(16):
        if c < 15:
            tri[c, : 15 - c] = 1.0
    neg = np.where(tri > 0, 0.0, NEG).astype(np.float32)
    zero16 = np.zeros((16, 15), np.float32)
    negall = np.full((16, 15), NEG, np.float32)

    per_core = []
    for rank in range(N_CORES):
        c0 = rank * SPAN
        w0 = np.zeros((P_ROWS, D_MODEL), np.float32)
        w0[:SPAN] = w_full[c0 : c0 + SPAN]
        blend = np.stack(
            [
                tri if rank == 0 else zero16,
                neg if rank == 0 else negall,
                tri if rank == N_CORES - 1 else zero16,
                neg if rank == N_CORES - 1 else negall,
            ]
        )
        per_core.append(
            {
                "w0": w0,
                "posT": posT,
                "wqkv": wqkv,
                "wr": wr,
                "wo": wo,
                "w1": w1,
                "w2": w2,
                "rwb": rwb,
                "rrb": rrb,
                "blend": np.ascontiguousarray(blend),
                "gb": gb,
                "b1col": b1col,
                "b2bc": b2bc,
            }
        )
    return per_core


def _layernorm(nc, sm, out_ap, x_ap, R, g, b, eps):
    f32 = DT.float32
    stats = sm.tile([128, 6], f32, tag="lnst")
    mv = sm.tile([128, 2], f32, tag="lnmv")
    nc.vector.bn_stats(stats[:R], x_ap)
    nc.vector.bn_aggr(mv[:R], stats[:R])
    std = sm.tile([128, 1], f32, tag="lnstd")
    nc.scalar.activation(std[:R], mv[:R, 1:2], AF.Sqrt, bias=eps, scale=1.0)
    rstd = sm.tile([128, 1], f32, tag="lnrstd")
    nc.vector.reciprocal(rstd[:R], std[:R])
    xn = sm.tile([128, D_MODEL], f32, tag="lnxn")
    if g is None:
        nc.vector.tensor_scalar(
            out=out_ap, in0=x_ap, scalar1=mv[:R, 0:1], scalar2=rstd[:R],
            op0=ALU.subtract, op1=ALU.mult,
        )
    else:
        nc.vector.tensor_scalar(
            out=xn[:R], in0=x_ap, scalar1=mv[:R, 0:1], scalar2=rstd[:R],
            op0=ALU.subtract, op1=ALU.mult,
        )
        nc.vector.tensor_tensor(xn[:R], xn[:R], g, ALU.mult)
        nc.vector.tensor_tensor(out_ap, xn[:R], b, ALU.add)


def _build(trivial_gb=True, trivial_b=True):
    nc = bacc.Bacc("TRN2", num_devices=N_CORES, dynamic_dma_scratch_size=4096)
    f32, bf16 = DT.float32, DT.bfloat16

    w0_t = nc.dram_tensor("w0", [P_ROWS, D_MODEL], f32, kind="ExternalInput")
    posT_t = nc.dram_tensor("posT", [D_MODEL, T], bf16, kind="ExternalInput")
    wqkv_t = nc.dram_tensor("wqkv", [N_LAYER, D_MODEL, 3 * D_MODEL], bf16, kind="ExternalInput")
    wr_t = nc.dram_tensor("wr", [N_LAYER, D_MODEL, D_MODEL], bf16, kind="ExternalInput")
    wo_t = nc.dram_tensor("wo", [N_LAYER, D_MODEL, D_MODEL], bf16, kind="ExternalInput")
    w1_t = nc.dram_tensor("w1", [N_LAYER, D_MODEL, D_INNER], bf16, kind="ExternalInput")
    w2_t = nc.dram_tensor("w2", [N_LAYER, D_INNER, D_MODEL], bf16, kind="ExternalInput")
    rwb_t = nc.dram_tensor("rwb", [D_MODEL, 1], f32, kind="ExternalInput")
    rrb_t = nc.dram_tensor("rrb", [D_MODEL, 1], f32, kind="ExternalInput")
    blend_t = nc.dram_tensor("blend", [4, 16, 15], f32, kind="ExternalInput")
    gb_t = None if trivial_gb else nc.dram_tensor(
        "gb", [N_LAYER, 128, 4, D_MODEL], f32, kind="ExternalInput")
    b1_t = b2_t = None
    if not trivial_b:
        b1_t = nc.dram_tensor("b1col", [N_LAYER, D_INNER, 1], f32, kind="ExternalInput")
        b2_t = nc.dram_tensor("b2bc", [N_LAYER, 128, D_MODEL], f32, kind="ExternalInput")
    out_t = nc.dram_tensor("wout", [SPAN, D_MODEL], f32, kind="ExternalOutput")

    # Per-head rel-shift buffers (+2 margin rows: the BD read AP over-claims).
    p_h = [
        nc.dram_tensor(f"pbuf{h}", [(P_ROWS + 2) * RW], bf16, kind="Internal")
        for h in range(N_HEAD)
    ]
    # views [part, qt, col] with row index qt*128+part at pitch RW
    p_epq = [
        p[0 : P_ROWS * RW].rearrange("(q p c) -> p q c", q=N_QT, p=128, c=RW)
        for p in p_h
    ]

    C_K = D_MODEL * SPAN
    C = 2 * C_K
    cag_in = nc.dram_tensor("cag_in", [C], bf16, kind="Internal")
    cag_out = nc.dram_tensor(
        "cag_out", [N_CORES * C], bf16, kind="Internal", addr_space="Shared"
    )
    cag_in_k = cag_in[0:C_K].rearrange("(d p c) -> p d c", d=HP, p=128, c=SPAN)
    rg = [list(range(N_CORES))]

    # batched weight views
    wqkv4 = wqkv_t.rearrange("l (dp p) n -> l p dp n", p=128)
    wr4 = wr_t.rearrange("l (dp p) n -> l p dp n", p=128)
    wo4 = wo_t.rearrange("l (dp p) n -> l p dp n", p=128)
    w14 = w1_t.rearrange("l (dp p) n -> l p dp n", p=128)
    w2c = w2_t.rearrange("l (c4 dp p) n -> l c4 p dp n", c4=4, p=128)
    posT4 = posT_t.rearrange("(dp p) c -> p dp c", p=128)

    with tile.TileContext(nc, num_cores=N_CORES) as tc:
        pid = nc.sync.partition_id()
        with (
            tc.tile_pool(name="const", bufs=1) as constp,
            tc.tile_pool(name="pers", bufs=1) as pers,
            tc.tile_pool(name="wts", bufs=1) as wts,
            tc.tile_pool(name="w2s", bufs=2) as w2s,
            tc.tile_pool(name="kv", bufs=1) as kvp,
            tc.tile_pool(name="eh", bufs=2) as ehp,
            tc.tile_pool(name="bdp", bufs=2) as bdp,
            tc.tile_pool(name="hot", bufs=2) as hot,
            tc.tile_pool(name="hot3", bufs=3) as hot3,
            tc.tile_pool(name="sm", bufs=2) as sm,
            tc.tile_pool(name="sm4", bufs=4) as sm4,
            tc.tile_pool(name="ps", bufs=4, space="PSUM") as ps,
            tc.tile_pool(name="pspv", bufs=1, space="PSUM") as pspv,
            tc.tile_pool(name="psff", bufs=1, space="PSUM") as psff,
        ):
            ident = constp.tile([128, 128], f32)
            make_identity(nc, ident[:])
            identb = constp.tile([128, 128], bf16)
            nc.vector.tensor_copy(identb[:], ident[:])
            blend_sb = constp.tile([16, 4, 15], f32)
            nc.sync.dma_start(
                blend_sb[:], blend_t.rearrange("a p c -> p a c")
            )
            eps_sb = constp.tile([128, 1], f32)
            nc.vector.memset(eps_sb[:], 1e-5)
            rwb_sb = constp.tile([128, HP], f32)
            rrb_sb = constp.tile([128, HP], f32)
            for d in range(HP):
                nc.sync.dma_start(rwb_sb[:, d : d + 1], rwb_t[d * 128 : (d + 1) * 128, :])
                nc.sync.dma_start(rrb_sb[:, d : d + 1], rrb_t[d * 128 : (d + 1) * 128, :])

            # init P buffers to -1e30 everywhere (mask region + pad rows);
            # issued on the scalar HWDGE ring so layer-0 weight loads on the
            # sync ring are not queued behind 25 MB of fill.
            negsmall = sm.tile([128, 2112], bf16, tag="scr4k")
            nc.vector.memset(negsmall[:], NEG)
            for h in range(N_HEAD):
                for q in range(N_QT):
                    nc.scalar.dma_start(p_epq[h][:, q, 0:2112], negsmall[:])
                    nc.scalar.dma_start(
                        p_epq[h][:, q, 2112:RW], negsmall[:, 0 : RW - 2112]
                    )
                tail = p_h[h][ds(P_ROWS * RW, 2 * RW)].rearrange(
                    "(r c) -> r c", c=RW
                )
                nc.scalar.dma_start(tail[:, 0:2112], negsmall[:2, :])
                nc.scalar.dma_start(tail[:, 2112:RW], negsmall[:2, 0 : RW - 2112])

            w_sb = pers.tile([128, N_QT, D_MODEL], f32)
            nc.sync.dma_start(
                w_sb[:],
                w0_t.rearrange("(q p) d -> p q d", p=128),
            )

            for l in range(N_LAYER):
                # ---- layer weights ----
                wqkv_sb = wts.tile([128, HP, 3 * D_MODEL], bf16, tag="wqkv")
                wr_sb = wts.tile([128, HP, D_MODEL], bf16, tag="wrl")
                wo_sb = wts.tile([128, HP, D_MODEL], bf16, tag="wol")
                w1_sb = wts.tile([128, HP, D_INNER], bf16, tag="w1l")
                nc.sync.dma_start(wqkv_sb[:], wqkv4[l])
                nc.sync.dma_start(wr_sb[:], wr4[l])
                nc.sync.dma_start(wo_sb[:], wo4[l])
                nc.sync.dma_start(w1_sb[:], w14[l])
                gb_sb = None
                if not trivial_gb:
                    gb_sb = wts.tile([128, 4, D_MODEL], f32, tag="gbl")
                    nc.sync.dma_start(gb_sb[:], gb_t[l])
                b1_sb = None
                b2_sb = None
                if not trivial_b:
                    b1_sb = wts.tile([128, 16], f32, tag="b1l")
                    for d in range(16):
                        nc.sync.dma_start(b1_sb[:, d : d + 1], b1_t[l, d * 128 : (d + 1) * 128, :])
                    b2_sb = wts.tile([128, D_MODEL], f32, tag="b2l")
                    nc.sync.dma_start(b2_sb[:], b2_t[l])

                # ---- transpose residual -> wT bf16 [128, hp, LR] ----
                wT_sb = wts.tile([128, HP, LR], bf16, tag="wT")
                for qt in range(N_QT):
                    W = QT_W[qt]
                    for d in range(HP):
                        pt = ps.tile([128, 512], f32, tag="big")
                        nc.tensor.transpose(
                            pt[:, :128], w_sb[:, qt, d * 128 : (d + 1) * 128], ident[:]
                        )
                        nc.scalar.copy(
                            wT_sb[:, d, qt * 128 : qt * 128 + W], pt[:, :W]
                        )

                # ---- projections ----
                qwT = wts.tile([128, HP, LR], bf16, tag="qwT")
                qrT = wts.tile([128, HP, LR], bf16, tag="qrT")
                kT_own = wts.tile([128, HP, SPAN], bf16, tag="kTown")
                for hp in range(HP):
                    pq = ps.tile([128, 512], f32, tag="big")
                    for d in range(HP):
                        nc.tensor.matmul(
                            pq[:, :LR],
                            wqkv_sb[:, d, hp * 128 : hp * 128 + 128],
                            wT_sb[:, d, :],
                            start=(d == 0), stop=(d == HP - 1),
                        )
                    nc.scalar.activation(
                        qwT[:, hp, :], pq[:, :LR], AF.Identity,
                        bias=rwb_sb[:, hp : hp + 1], scale=1.0,
                    )
                    nc.scalar.activation(
                        qrT[:, hp, :], pq[:, :LR], AF.Identity,
                        bias=rrb_sb[:, hp : hp + 1], scale=1.0,
                    )
                    pk = ps.tile([128, 512], f32, tag="big")
                    for d in range(HP):
                        nc.tensor.matmul(
                            pk[:, :LR],
                            wqkv_sb[:, d, D_MODEL + hp * 128 : D_MODEL + hp * 128 + 128],
                            wT_sb[:, d, :],
                            start=(d == 0), stop=(d == HP - 1),
                        )
                    nc.scalar.copy(kT_own[:, hp, :], pk[:, :SPAN])
                nc.sync.dma_start(cag_in_k, kT_own[:])

                v_own = sm.tile([128, N_QT, D_MODEL], bf16, tag="scr4k")
                for qt in range(N_QT):
                    W = QT_W[qt]
                    pv = ps.tile([128, 512], f32, tag="big")
                    for d in range(HP):
                        nc.tensor.matmul(
                            pv[:W, :D_MODEL],
                            wT_sb[:, d, qt * 128 : qt * 128 + W],
                            wqkv_sb[:, d, 2 * D_MODEL :],
                            start=(d == 0), stop=(d == HP - 1),
                        )
                    nc.vector.tensor_copy(v_own[:W, qt, :], pv[:W, :D_MODEL])
                    rows = QT_REAL[qt]
                    vtgt = cag_in[
                        C_K + (qt * 128) * D_MODEL : C_K + (qt * 128 + rows) * D_MODEL
                    ].rearrange("(r d) -> r d", d=D_MODEL)
                    nc.sync.dma_start(vtgt, v_own[:rows, qt, :])

                nc.gpsimd.collective_compute(
                    "AllGather", ALU.bypass, replica_groups=rg,
                    ins=[cag_in[:]], outs=[cag_out[:]],
                )

                # ---- r^T (posT streamed from DRAM), K/V-independent ----
                rT_sb = wts.tile([128, HP, T], bf16, tag="rT")
                for ch in range(5):
                    cw = min(512, T - ch * 512)
                    pos_big = sm.tile([128, HP, 512], bf16, tag="scr4k")
                    nc.sync.dma_start(
                        pos_big[:, :, :cw], posT4[:, :, ch * 512 : ch * 512 + cw]
                    )
                    for hp in range(HP):
                        pr = ps.tile([128, 512], f32, tag="big")
                        for d in range(HP):
                            nc.tensor.matmul(
                                pr[:, :cw],
                                wr_sb[:, d, hp * 128 : hp * 128 + 128],
                                pos_big[:, d, :cw],
                                start=(d == 0), stop=(d == HP - 1),
                            )
                        nc.scalar.copy(
                            rT_sb[:, hp, ch * 512 : ch * 512 + cw], pr[:, :cw]
                        )

                # ---- E pass (writes rel-shift buffers; overlaps AllGather) ----
                # Heads are processed in pairs: head 2hp uses PE rows 0-63,
                # head 2hp+1 rows 64-127, so their K=64 matmuls overlap in
                # the array (per-subarray row-group concurrency).
                for hp in range(HP):
                    e0 = ehp.tile([128, N_QT, T], bf16, tag="eh")
                    e1 = ehp.tile([128, N_QT, T], bf16, tag="eh")
                    e_pair = [e0, e1]
                    for qt in range(N_QT):
                        W = QT_W[qt]
                        qsl = slice(qt * 128, qt * 128 + W)
                        for cc in range(N_CC):
                            cw = CC_W[cc]
                            jsl = slice(cc * 512, cc * 512 + cw)
                            pe0 = ps.tile([128, 512], f32, tag="big")
                            pe1 = ps.tile([128, 512], f32, tag="big")
                            nc.tensor.matmul(
                                pe0[:W, :cw],
                                qrT[0:64, hp, qsl],
                                rT_sb[0:64, hp, jsl],
                                start=True, stop=True,
                            )
                            nc.tensor.matmul(
                                pe1[:W, :cw],
                                qrT[64:128, hp, qsl],
                                rT_sb[64:128, hp, jsl],
                                start=True, stop=True,
                            )
                            nc.vector.tensor_copy(e0[:W, qt, jsl], pe0[:W, :cw])
                            nc.vector.tensor_copy(e1[:W, qt, jsl], pe1[:W, :cw])
                    for hh in range(2):
                        h = 2 * hp + hh
                        par = hh * 64
                        e_sb = e_pair[hh]
                        # write E region (rows qt*128+r, cols [1, T+1))
                        nc.sync.dma_start(
                            p_epq[h][:, 0:2, 1 : 1 + T], e_sb[:, 0:2, :]
                        )
                        nc.sync.dma_start(
                            p_epq[h][:16, 2, 1 : 1 + T], e_sb[:16, 2, :]
                        )
                        # corner patches: raw E rows (c+1) cols [0,14) blended
                        for cn in range(2):
                            src0 = 1 if cn == 0 else 245
                            pcs = ps.tile([128, 512], f32, tag="big")
                            nc.tensor.matmul(
                                pcs[:16, :14],
                                qrT[par : par + 64, hp, src0 : src0 + 16],
                                rT_sb[par : par + 64, hp, 0:14],
                                start=True, stop=True,
                            )
                            patch = sm.tile([16, 15], f32, tag="patch")
                            nc.vector.memset(patch[:], 0.0)
                            nc.vector.tensor_copy(patch[:, 1:15], pcs[:16, :14])
                            nc.vector.tensor_tensor(
                                patch[:], patch[:], blend_sb[:, 2 * cn, :], ALU.mult
                            )
                            patchb = sm.tile([16, 15], bf16, tag="patchb")
                            nc.vector.tensor_tensor(
                                patchb[:], patch[:], blend_sb[:, 2 * cn + 1, :], ALU.add
                            )
                            row0 = 0 if cn == 0 else 244
                            tgt = p_h[h][
                                ds(row0 * RW + (T + 1), 16 * RW)
                            ].rearrange("(r c) -> r c", c=RW)[0:16, 0:15]
                            nc.sync.dma_start(tgt, patchb[:])

                # ---- gathered K/V into SBUF ----
                kT_all = kvp.tile([128, HP, T], bf16, tag="kTall")
                for r in range(N_CORES):
                    src = cag_out[r * C : r * C + C_K].rearrange(
                        "(d p c) -> p d c", d=HP, p=128, c=SPAN
                    )
                    nc.sync.dma_start(
                        kT_all[:, :, r * SPAN : (r + 1) * SPAN], src
                    )
                v_all = kvp.tile([128, 17, D_MODEL], bf16, tag="vall")
                nc.vector.memset(v_all[:, 16, :], 0.0)
                for r in range(N_CORES):
                    g0 = r * SPAN
                    rem = SPAN
                    src_off = r * C + C_K
                    while rem > 0:
                        t_i, p0 = g0 // 128, g0 % 128
                        cnt = min(128 - p0, rem)
                        src = cag_out[
                            src_off : src_off + cnt * D_MODEL
                        ].rearrange("(r d) -> r d", d=D_MODEL)
                        nc.sync.dma_start(v_all[p0 : p0 + cnt, t_i, :], src)
                        g0 += cnt
                        rem -= cnt
                        src_off += cnt * D_MODEL

                # ---- attention pass B ----
                attnT = wts.tile([128, HP, LR], bf16, tag="attnT")
                for hp in range(HP):
                    # head pair processed interleaved: AC matmuls of head
                    # 2hp (rows 0-63) and 2hp+1 (rows 64-127) overlap in PE
                    off = T - pid * SPAN
                    bd_pair = []
                    probT_pair = []
                    for hh in range(2):
                        h = 2 * hp + hh
                        bd_sb = bdp.tile([128, N_QT, T], bf16, tag="bd")
                        src = p_h[h][ds(off, N_QT * 128 * 2 * T)].rearrange(
                            "(q p c) -> p q c", q=N_QT, p=128, c=2 * T
                        )[:, :, 0:T]
                        nc.sync.dma_start(bd_sb[:], src)
                        bd_pair.append(bd_sb)
                        probT_pair.append(
                            hot.tile([128, 17, LR], bf16, tag="probT",
                                     name=f"probT{hh}")
                        )
                    for qt in range(N_QT):
                        W = QT_W[qt]
                        qsl = slice(qt * 128, qt * 128 + W)
                        prob_pair = [
                            hot3.tile([128, TPAD], bf16, tag="prob",
                                      name=f"prob{i}")
                            for i in range(2)
                        ]
                        dens_pair = [
                            sm4.tile([128, 8], f32, tag="dens", name=f"dens{i}")
                            for i in range(2)
                        ]
                        for cc in range(N_CC):
                            cw = CC_W[cc]
                            jsl = slice(cc * 512, cc * 512 + cw)
                            pa0 = ps.tile([128, 512], f32, tag="big")
                            pa1 = ps.tile([128, 512], f32, tag="big")
                            pa_pair = [pa0, pa1]
                            for hh in range(2):
                                par = hh * 64
                                nc.tensor.matmul(
                                    pa_pair[hh][:W, :cw],
                                    qwT[par : par + 64, hp, qsl],
                                    kT_all[par : par + 64, hp, jsl],
                                    start=True, stop=False,
                                )
                            for hh in range(2):
                                nc.tensor.matmul(
                                    pa_pair[hh][:W, :cw],
                                    identb[:W, :W],
                                    bd_pair[hh][:W, qt, jsl],
                                    start=False, stop=True,
                                )
                            for hh in range(2):
                                nc.scalar.activation(
                                    prob_pair[hh][:W, jsl],
                                    pa_pair[hh][:W, :cw],
                                    AF.Exp, bias=0.0, scale=float(SCALE),
                                    accum_out=dens_pair[hh][:W, cc : cc + 1],
                                )
                        for hh in range(2):
                            prob = prob_pair[hh]
                            nc.vector.memset(prob[:W, T:], 0.0)
                            den = sm4.tile([128, 2], f32, tag="den")
                            nc.vector.tensor_reduce(
                                den[:W, 0:1], dens_pair[hh][:W, 0:N_CC],
                                axis=mybir.AxisListType.X, op=ALU.add,
                            )
                            rden = sm4.tile([128, 1], f32, tag="rden")
                            nc.vector.reciprocal(rden[:W], den[:W, 0:1])
                            nc.vector.tensor_scalar(
                                out=prob[:W, :], in0=prob[:W, :],
                                scalar1=rden[:W], scalar2=None, op0=ALU.mult,
                            )
                            nc.sync.dma_start_transpose(
                                probT_pair[hh][:, :, qt * 128 : qt * 128 + W],
                                prob[:W, :],
                            )
                    for hh in range(2):
                        h = 2 * hp + hh
                        par = hh * 64
                        ppv = pspv.tile([64, LR], f32, tag="ppv")
                        for t_i in range(17):
                            nc.tensor.matmul(
                                ppv[:],
                                v_all[:, t_i, h * 64 : h * 64 + 64],
                                probT_pair[hh][:, t_i, :],
                                start=(t_i == 0), stop=(t_i == 16),
                            )
                        nc.scalar.copy(attnT[par : par + 64, hp, :], ppv[:])

                # ---- Wo + residual + LN1 ----
                for qt in range(N_QT):
                    W = QT_W[qt]
                    pw = ps.tile([128, 512], f32, tag="big")
                    for d in range(HP):
                        nc.tensor.matmul(
                            pw[:W, :D_MODEL],
                            attnT[:, d, qt * 128 : qt * 128 + W],
                            wo_sb[:, d, :],
                            start=(d == 0), stop=(d == HP - 1),
                        )
                    x = sm.tile([128, D_MODEL], f32, tag="xres")
                    nc.vector.tensor_tensor(
                        x[:W], w_sb[:W, qt, :], pw[:W, :D_MODEL], ALU.add
                    )
                    _layernorm(
                        nc, sm, w_sb[:W, qt, :], x[:W], W,
                        None if trivial_gb else gb_sb[:W, 0, :],
                        None if trivial_gb else gb_sb[:W, 1, :],
                        eps_sb[:W],
                    )

                # ---- FFN ----
                w1T = wts.tile([128, HP, LR], bf16, tag="w1T")
                for qt in range(N_QT):
                    W = QT_W[qt]
                    for d in range(HP):
                        pt = ps.tile([128, 512], f32, tag="big")
                        nc.tensor.transpose(
                            pt[:, :128], w_sb[:, qt, d * 128 : (d + 1) * 128], ident[:]
                        )
                        nc.scalar.copy(
                            w1T[:, d, qt * 128 : qt * 128 + W], pt[:, :W]
                        )
                pf = [
                    psff.tile([128, 512], f32, tag=f"pf{qt}", name=f"pf{qt}")
                    for qt in range(N_QT)
                ]
                for dc in range(4):
                    w2_sb = w2s.tile([128, HP, D_MODEL], bf16, tag="w2l")
                    nc.sync.dma_start(w2_sb[:], w2c[l, dc])
                    for di4 in range(4):
                        di = dc * 4 + di4
                        phh = ps.tile([128, 512], f32, tag="big")
                        for d in range(HP):
                            nc.tensor.matmul(
                                phh[:, :LR],
                                w1_sb[:, d, di * 128 : (di + 1) * 128],
                                w1T[:, d, :],
                                start=(d == 0), stop=(d == HP - 1),
                            )
                        h1t = sm.tile([128, LR], bf16, tag="h1t")
                        if trivial_b:
                            nc.scalar.activation(
                                h1t[:], phh[:, :LR], AF.Relu, bias=0.0, scale=1.0
                            )
                        else:
                            nc.scalar.activation(
                                h1t[:], phh[:, :LR], AF.Relu,
                                bias=b1_sb[:, di : di + 1], scale=1.0,
                            )
                        for qt in range(N_QT):
                            W = QT_W[qt]
                            nc.tensor.matmul(
                                pf[qt][:W],
                                h1t[:, qt * 128 : qt * 128 + W],
                                w2_sb[:, di4, :],
                                start=(di == 0), stop=(di == 15),
                            )
                for qt in range(N_QT):
                    W = QT_W[qt]
                    x = sm.tile([128, D_MODEL], f32, tag="xres")
                    if trivial_b:
                        nc.vector.tensor_tensor(
                            x[:W], pf[qt][:W], w_sb[:W, qt, :], ALU.add
                        )
                    else:
                        nc.vector.scalar_tensor_tensor(
                            x[:W], pf[qt][:W], 1.0, b2_sb[:W], ALU.mult, ALU.add
                        )
                        nc.vector.tensor_tensor(x[:W], x[:W], w_sb[:W, qt, :], ALU.add)
                    _layernorm(
                        nc, sm, w_sb[:W, qt, :], x[:W], W,
                        None if trivial_gb else gb_sb[:W, 2, :],
                        None if trivial_gb else gb_sb[:W, 3, :],
                        eps_sb[:W],
                    )

            for qt in range(N_QT):
                rows = QT_REAL[qt]
                nc.sync.dma_start(
                    out_t[qt * 128 : qt * 128 + rows, :], w_sb[:rows, qt, :]
                )

    nc.compile()
    return nc


_NC_CACHE = {}
LAST_RESULT = None


def kernel(**inputs):
    trivial_gb = (
        np.all(np.asarray(inputs["ln1_scale"]) == 1.0)
        and np.all(np.asarray(inputs["ln2_scale"]) == 1.0)
        and np.all(np.asarray(inputs["ln1_bias"]) == 0.0)
        and np.all(np.asarray(inputs["ln2_bias"]) == 0.0)
    )
    trivial_b = (
        np.all(np.asarray(inputs["ffn_b1"]) == 0.0)
        and np.all(np.asarray(inputs["ffn_b2"]) == 0.0)
    )
    per_core = _host_prep(inputs)
    drop = []
    if trivial_gb:
        drop.append("gb")
    if trivial_b:
        drop += ["b1col", "b2bc"]
    for pc in per_core:
        for k in drop:
            pc.pop(k, None)
    key = (trivial_gb, trivial_b)
    if key not in _NC_CACHE:
        _NC_CACHE[key] = _build(trivial_gb=trivial_gb, trivial_b=trivial_b)
    res = run_bass_kernel_spmd(
        _NC_CACHE[key], [dict(pc) for pc in per_core], core_ids=list(range(N_CORES))
    )
    global LAST_RESULT
    LAST_RESULT = res
    spans = [res.results[r]["wout"] for r in range(N_CORES)]
    out = np.concatenate(spans, axis=0)
    return np.ascontiguousarray(out[:, None, :].astype(np.float32))


# revision 19
# speedup vs baseline: 1.0228x; 1.0228x over previous
"""Trainium2 Bass kernel for nn_MemTransformerLM (Transformer-XL rel-attention).

Sequence-parallel over 8 NeuronCores: core m owns token span [260m, 260(m+1)).

v2 restructure vs baseline:
- Per-head DRAM rel-shift buffers with row pitch 2T+1: each row is
  [zero][E (T)][mask (T)], where the mask region holds -1e30 so the
  Transformer-XL shift's wrap-read lands on the additive causal mask for
  free (no on-chip mask add). Mem-token corner "garbage" (which the
  reference keeps) is reproduced by patching the first 15 mask cells of the
  32 corner rows each layer with raw E values via tiny matmuls + blends
  driven by per-core blend masks (identity off the corner cores).
- BD is accumulated into the AC PSUM via an identity matmul, and softmax's
  exp reads PSUM directly with the 1/sqrt(d) scale folded into the
  activation, eliminating the separate mask-add/score ops.
- One combined K+V AllGather per layer, overlapped with the (K/V
  independent) E pass.
- Batched 3D-AP DMAs for weights, posT, K/V gather, and E/BD round trips.
"""

import numpy as np
import ml_dtypes

import concourse.bass as bass
import concourse.mybir as mybir
import concourse.tile as tile
from concourse import bacc
from concourse.bass import ds
from concourse.bass_utils import run_bass_kernel_spmd
from concourse.masks import make_identity

BF16 = ml_dtypes.bfloat16
DT = mybir.dt
AF = mybir.ActivationFunctionType
ALU = mybir.AluOpType

N_LAYER = 4
N_HEAD = 8
D_HEAD = 64
D_MODEL = 512
D_INNER = 2048
NMT = 16
T = 2048 + 2 * NMT      # 2080
N_CORES = 8
SPAN = T // N_CORES     # 260
QT_REAL = [128, 128, 4]
QT_W = [128, 128, 16]   # on-chip partition width per q-tile (>=16 for xbar)
N_QT = 3
LR = 272                # local-row extent (2*128 + 16)
NEG = -1e30
SCALE = 1.0 / np.sqrt(D_HEAD)
TPAD = 17 * 128         # 2176
RW = 2 * T + 1          # P-buffer row pitch: [zero][E(T)][M(T)]
P_ROWS = 384
CC_W = [512, 512, 512, 512, 32]  # exp/psum chunking of the j axis
N_CC = 5
HP = 4                  # head-pair partition tiles


def _host_prep(inputs):
    word_emb = np.asarray(inputs["word_emb"], np.float32)
    mem_tokens = np.asarray(inputs["mem_tokens"], np.float32)
    w = np.transpose(word_emb, (1, 0, 2))[:, 0, :]
    mem = mem_tokens[:, 0, :]
    w_full = np.concatenate([mem, w, mem], axis=0)          # [T, 512]

    inv_freq = 1.0 / (10000.0 ** (np.arange(0, D_MODEL, 2, dtype=np.float32) / D_MODEL))
    pos_seq = np.arange(T - 1, -1, -1.0, dtype=np.float32)
    sinusoid = pos_seq[:, None] * inv_freq[None, :]
    pos_emb = np.concatenate([np.sin(sinusoid), np.cos(sinusoid)], axis=-1)
    posT = np.ascontiguousarray(pos_emb.T).astype(BF16)     # [512, T]

    wqkv = np.asarray(inputs["Wqkv"], np.float32).astype(BF16)
    wr = np.asarray(inputs["Wr"], np.float32).astype(BF16)
    wo = np.asarray(inputs["Wo"], np.float32).astype(BF16)
    w1 = np.asarray(inputs["ffn_W1"], np.float32).astype(BF16)
    w2 = np.asarray(inputs["ffn_W2"], np.float32).astype(BF16)
    rwb = np.ascontiguousarray(np.asarray(inputs["r_w_bias"], np.float32).reshape(-1, 1))
    rrb = np.ascontiguousarray(np.asarray(inputs["r_r_bias"], np.float32).reshape(-1, 1))

    ln1g = np.asarray(inputs["ln1_scale"], np.float32)
    ln1b = np.asarray(inputs["ln1_bias"], np.float32)
    ln2g = np.asarray(inputs["ln2_scale"], np.float32)
    ln2b = np.asarray(inputs["ln2_bias"], np.float32)
    gb = np.ascontiguousarray(
        np.stack(
            [
                np.broadcast_to(ln1g[:, None, :], (N_LAYER, 128, D_MODEL)),
                np.broadcast_to(ln1b[:, None, :], (N_LAYER, 128, D_MODEL)),
                np.broadcast_to(ln2g[:, None, :], (N_LAYER, 128, D_MODEL)),
                np.broadcast_to(ln2b[:, None, :], (N_LAYER, 128, D_MODEL)),
            ],
            axis=2,
        ).astype(np.float32)
    )
    b1col = np.ascontiguousarray(
        np.asarray(inputs["ffn_b1"], np.float32).reshape(N_LAYER, D_INNER, 1)
    )
    b2bc = np.ascontiguousarray(
        np.broadcast_to(
            np.asarray(inputs["ffn_b2"], np.float32)[:, None, :],
            (N_LAYER, 128, D_MODEL),
        ).copy()
    )

    # Corner blend masks: tri[c, y] selects the wrap-garbage region
    # (y < 15-c) of the patched mask cells; neg fills the rest with -1e30.
    tri = np.zeros((16, 15), np.float32)
    for c in range(16):
        if c < 15:
            tri[c, : 15 - c] = 1.0
    neg = np.where(tri > 0, 0.0, NEG).astype(np.float32)
    zero16 = np.zeros((16, 15), np.float32)
    negall = np.full((16, 15), NEG, np.float32)

    per_core = []
    for rank in range(N_CORES):
        c0 = rank * SPAN
        w0 = np.zeros((P_ROWS, D_MODEL), np.float32)
        w0[:SPAN] = w_full[c0 : c0 + SPAN]
        blend = np.stack(
            [
                tri if rank == 0 else zero16,
                neg if rank == 0 else negall,
                tri if rank == N_CORES - 1 else zero16,
                neg if rank == N_CORES - 1 else negall,
            ]
        )
        per_core.append(
            {
                "w0": w0,
                "posT": posT,
                "wqkv": wqkv,
                "wr": wr,
                "wo": wo,
                "w1": w1,
                "w2": w2,
                "rwb": rwb,
                "rrb": rrb,
                "blend": np.ascontiguousarray(blend),
                "gb": gb,
                "b1col": b1col,
                "b2bc": b2bc,
            }
        )
    return per_core


def _layernorm(nc, sm, out_ap, x_ap, R, g, b, eps):
    f32 = DT.float32
    stats = sm.tile([128, 6], f32, tag="lnst")
    mv = sm.tile([128, 2], f32, tag="lnmv")
    nc.vector.bn_stats(stats[:R], x_ap)
    nc.vector.bn_aggr(mv[:R], stats[:R])
    std = sm.tile([128, 1], f32, tag="lnstd")
    nc.scalar.activation(std[:R], mv[:R, 1:2], AF.Sqrt, bias=eps, scale=1.0)
    rstd = sm.tile([128, 1], f32, tag="lnrstd")
    nc.vector.reciprocal(rstd[:R], std[:R])
    xn = sm.tile([128, D_MODEL], f32, tag="lnxn")
    if g is None:
        nc.vector.tensor_scalar(
            out=out_ap, in0=x_ap, scalar1=mv[:R, 0:1], scalar2=rstd[:R],
            op0=ALU.subtract, op1=ALU.mult,
        )
    else:
        nc.vector.tensor_scalar(
            out=xn[:R], in0=x_ap, scalar1=mv[:R, 0:1], scalar2=rstd[:R],
            op0=ALU.subtract, op1=ALU.mult,
        )
        nc.vector.tensor_tensor(xn[:R], xn[:R], g, ALU.mult)
        nc.vector.tensor_tensor(out_ap, xn[:R], b, ALU.add)


def _build(trivial_gb=True, trivial_b=True):
    nc = bacc.Bacc("TRN2", num_devices=N_CORES, dynamic_dma_scratch_size=4096)
    f32, bf16 = DT.float32, DT.bfloat16

    w0_t = nc.dram_tensor("w0", [P_ROWS, D_MODEL], f32, kind="ExternalInput")
    posT_t = nc.dram_tensor("posT", [D_MODEL, T], bf16, kind="ExternalInput")
    wqkv_t = nc.dram_tensor("wqkv", [N_LAYER, D_MODEL, 3 * D_MODEL], bf16, kind="ExternalInput")
    wr_t = nc.dram_tensor("wr", [N_LAYER, D_MODEL, D_MODEL], bf16, kind="ExternalInput")
    wo_t = nc.dram_tensor("wo", [N_LAYER, D_MODEL, D_MODEL], bf16, kind="ExternalInput")
    w1_t = nc.dram_tensor("w1", [N_LAYER, D_MODEL, D_INNER], bf16, kind="ExternalInput")
    w2_t = nc.dram_tensor("w2", [N_LAYER, D_INNER, D_MODEL], bf16, kind="ExternalInput")
    rwb_t = nc.dram_tensor("rwb", [D_MODEL, 1], f32, kind="ExternalInput")
    rrb_t = nc.dram_tensor("rrb", [D_MODEL, 1], f32, kind="ExternalInput")
    blend_t = nc.dram_tensor("blend", [4, 16, 15], f32, kind="ExternalInput")
    gb_t = None if trivial_gb else nc.dram_tensor(
        "gb", [N_LAYER, 128, 4, D_MODEL], f32, kind="ExternalInput")
    b1_t = b2_t = None
    if not trivial_b:
        b1_t = nc.dram_tensor("b1col", [N_LAYER, D_INNER, 1], f32, kind="ExternalInput")
        b2_t = nc.dram_tensor("b2bc", [N_LAYER, 128, D_MODEL], f32, kind="ExternalInput")
    out_t = nc.dram_tensor("wout", [SPAN, D_MODEL], f32, kind="ExternalOutput")

    # Per-head rel-shift buffers (+2 margin rows: the BD read AP over-claims).
    p_h = [
        nc.dram_tensor(f"pbuf{h}", [(P_ROWS + 2) * RW], bf16, kind="Internal")
        for h in range(N_HEAD)
    ]
    # views [part, qt, col] with row index qt*128+part at pitch RW
    p_epq = [
        p[0 : P_ROWS * RW].rearrange("(q p c) -> p q c", q=N_QT, p=128, c=RW)
        for p in p_h
    ]

    C_K = D_MODEL * SPAN
    C = 2 * C_K
    cag_in = nc.dram_tensor("cag_in", [C], bf16, kind="Internal")
    cag_out = nc.dram_tensor(
        "cag_out", [N_CORES * C], bf16, kind="Internal", addr_space="Shared"
    )
    cag_in_k = cag_in[0:C_K].rearrange("(d p c) -> p d c", d=HP, p=128, c=SPAN)
    rg = [list(range(N_CORES))]

    # batched weight views
    wqkv4 = wqkv_t.rearrange("l (dp p) n -> l p dp n", p=128)
    wr4 = wr_t.rearrange("l (dp p) n -> l p dp n", p=128)
    wo4 = wo_t.rearrange("l (dp p) n -> l p dp n", p=128)
    w14 = w1_t.rearrange("l (dp p) n -> l p dp n", p=128)
    w2c = w2_t.rearrange("l (c4 dp p) n -> l c4 p dp n", c4=4, p=128)
    posT4 = posT_t.rearrange("(dp p) c -> p dp c", p=128)

    with tile.TileContext(nc, num_cores=N_CORES) as tc:
        pid = nc.sync.partition_id()
        with (
            tc.tile_pool(name="const", bufs=1) as constp,
            tc.tile_pool(name="pers", bufs=1) as pers,
            tc.tile_pool(name="wts", bufs=1) as wts,
            tc.tile_pool(name="w2s", bufs=2) as w2s,
            tc.tile_pool(name="kv", bufs=1) as kvp,
            tc.tile_pool(name="eh", bufs=2) as ehp,
            tc.tile_pool(name="bdp", bufs=2) as bdp,
            tc.tile_pool(name="hot", bufs=2) as hot,
            tc.tile_pool(name="hot3", bufs=3) as hot3,
            tc.tile_pool(name="sm", bufs=2) as sm,
            tc.tile_pool(name="sm4", bufs=4) as sm4,
            tc.tile_pool(name="ps", bufs=4, space="PSUM") as ps,
            tc.tile_pool(name="pspv", bufs=1, space="PSUM") as pspv,
            tc.tile_pool(name="psff", bufs=1, space="PSUM") as psff,
        ):
            ident = constp.tile([128, 128], f32)
            make_identity(nc, ident[:])
            identb = constp.tile([128, 128], bf16)
            nc.vector.tensor_copy(identb[:], ident[:])
            blend_sb = constp.tile([16, 4, 15], f32)
            nc.sync.dma_start(
                blend_sb[:], blend_t.rearrange("a p c -> p a c")
            )
            eps_sb = constp.tile([128, 1], f32)
            nc.vector.memset(eps_sb[:], 1e-5)
            rwb_sb = constp.tile([128, HP], f32)
            rrb_sb = constp.tile([128, HP], f32)
            for d in range(HP):
                nc.sync.dma_start(rwb_sb[:, d : d + 1], rwb_t[d * 128 : (d + 1) * 128, :])
                nc.sync.dma_start(rrb_sb[:, d : d + 1], rrb_t[d * 128 : (d + 1) * 128, :])

            # init P buffers to -1e30 everywhere (mask region + pad rows);
            # issued on the scalar HWDGE ring so layer-0 weight loads on the
            # sync ring are not queued behind 25 MB of fill.
            negsmall = sm.tile([128, 2112], bf16, tag="scr4k")
            nc.vector.memset(negsmall[:], NEG)
            for h in range(N_HEAD):
                for q in range(N_QT):
                    nc.scalar.dma_start(p_epq[h][:, q, 0:2112], negsmall[:])
                    nc.scalar.dma_start(
                        p_epq[h][:, q, 2112:RW], negsmall[:, 0 : RW - 2112]
                    )
                tail = p_h[h][ds(P_ROWS * RW, 2 * RW)].rearrange(
                    "(r c) -> r c", c=RW
                )
                nc.scalar.dma_start(tail[:, 0:2112], negsmall[:2, :])
                nc.scalar.dma_start(tail[:, 2112:RW], negsmall[:2, 0 : RW - 2112])

            w_sb = pers.tile([128, N_QT, D_MODEL], f32)
            nc.sync.dma_start(
                w_sb[:],
                w0_t.rearrange("(q p) d -> p q d", p=128),
            )

            for l in range(N_LAYER):
                # ---- layer weights ----
                wqkv_sb = wts.tile([128, HP, 3 * D_MODEL], bf16, tag="wqkv")
                wr_sb = wts.tile([128, HP, D_MODEL], bf16, tag="wrl")
                wo_sb = wts.tile([128, HP, D_MODEL], bf16, tag="wol")
                w1_sb = wts.tile([128, HP, D_INNER], bf16, tag="w1l")
                nc.sync.dma_start(wqkv_sb[:], wqkv4[l])
                nc.sync.dma_start(wr_sb[:], wr4[l])
                nc.sync.dma_start(wo_sb[:], wo4[l])
                nc.sync.dma_start(w1_sb[:], w14[l])
                gb_sb = None
                if not trivial_gb:
                    gb_sb = wts.tile([128, 4, D_MODEL], f32, tag="gbl")
                    nc.sync.dma_start(gb_sb[:], gb_t[l])
                b1_sb = None
                b2_sb = None
                if not trivial_b:
                    b1_sb = wts.tile([128, 16], f32, tag="b1l")
                    for d in range(16):
                        nc.sync.dma_start(b1_sb[:, d : d + 1], b1_t[l, d * 128 : (d + 1) * 128, :])
                    b2_sb = wts.tile([128, D_MODEL], f32, tag="b2l")
                    nc.sync.dma_start(b2_sb[:], b2_t[l])

                # ---- transpose residual -> wT bf16 [128, hp, LR] ----
                wT_sb = wts.tile([128, HP, LR], bf16, tag="wT")
                for qt in range(N_QT):
                    W = QT_W[qt]
                    for d in range(HP):
                        pt = ps.tile([128, 512], f32, tag="big")
                        nc.tensor.transpose(
                            pt[:, :128], w_sb[:, qt, d * 128 : (d + 1) * 128], ident[:]
                        )
                        nc.scalar.copy(
                            wT_sb[:, d, qt * 128 : qt * 128 + W], pt[:, :W]
                        )

                # ---- projections ----
                qwT = wts.tile([128, HP, LR], bf16, tag="qwT")
                qrT = wts.tile([128, HP, LR], bf16, tag="qrT")
                kT_own = wts.tile([128, HP, SPAN], bf16, tag="kTown")
                for hp in range(HP):
                    pq = ps.tile([128, 512], f32, tag="big")
                    for d in range(HP):
                        nc.tensor.matmul(
                            pq[:, :LR],
                            wqkv_sb[:, d, hp * 128 : hp * 128 + 128],
                            wT_sb[:, d, :],
                            start=(d == 0), stop=(d == HP - 1),
                        )
                    nc.scalar.activation(
                        qwT[:, hp, :], pq[:, :LR], AF.Identity,
                        bias=rwb_sb[:, hp : hp + 1], scale=1.0,
                    )
                    nc.scalar.activation(
                        qrT[:, hp, :], pq[:, :LR], AF.Identity,
                        bias=rrb_sb[:, hp : hp + 1], scale=1.0,
                    )
                    pk = ps.tile([128, 512], f32, tag="big")
                    for d in range(HP):
                        nc.tensor.matmul(
                            pk[:, :LR],
                            wqkv_sb[:, d, D_MODEL + hp * 128 : D_MODEL + hp * 128 + 128],
                            wT_sb[:, d, :],
                            start=(d == 0), stop=(d == HP - 1),
                        )
                    nc.scalar.copy(kT_own[:, hp, :], pk[:, :SPAN])
                nc.sync.dma_start(cag_in_k, kT_own[:])

                v_own = sm.tile([128, N_QT, D_MODEL], bf16, tag="scr4k")
                for qt in range(N_QT):
                    W = QT_W[qt]
                    pv = ps.tile([128, 512], f32, tag="big")
                    for d in range(HP):
                        nc.tensor.matmul(
                            pv[:W, :D_MODEL],
                            wT_sb[:, d, qt * 128 : qt * 128 + W],
                            wqkv_sb[:, d, 2 * D_MODEL :],
                            start=(d == 0), stop=(d == HP - 1),
                        )
                    nc.vector.tensor_copy(v_own[:W, qt, :], pv[:W, :D_MODEL])
                    rows = QT_REAL[qt]
                    vtgt = cag_in[
                        C_K + (qt * 128) * D_MODEL : C_K + (qt * 128 + rows) * D_MODEL
                    ].rearrange("(r d) -> r d", d=D_MODEL)
                    nc.sync.dma_start(vtgt, v_own[:rows, qt, :])

                nc.gpsimd.collective_compute(
                    "AllGather", ALU.bypass, replica_groups=rg,
                    ins=[cag_in[:]], outs=[cag_out[:]],
                )

                # ---- r^T (posT streamed from DRAM), K/V-independent ----
                rT_sb = wts.tile([128, HP, T], bf16, tag="rT")
                for ch in range(5):
                    cw = min(512, T - ch * 512)
                    pos_big = sm.tile([128, HP, 512], bf16, tag="scr4k")
                    nc.sync.dma_start(
                        pos_big[:, :, :cw], posT4[:, :, ch * 512 : ch * 512 + cw]
                    )
                    for hp in range(HP):
                        pr = ps.tile([128, 512], f32, tag="big")
                        for d in range(HP):
                            nc.tensor.matmul(
                                pr[:, :cw],
                                wr_sb[:, d, hp * 128 : hp * 128 + 128],
                                pos_big[:, d, :cw],
                                start=(d == 0), stop=(d == HP - 1),
                            )
                        nc.scalar.copy(
                            rT_sb[:, hp, ch * 512 : ch * 512 + cw], pr[:, :cw]
                        )

                # ---- E pass (writes rel-shift buffers; overlaps AllGather) ----
                for hp in range(HP):
                    for hh in range(2):
                        par = hh * 64
                        e_sb = ehp.tile([128, N_QT, T], bf16, tag="eh")
                        for qt in range(N_QT):
                            W = QT_W[qt]
                            qsl = slice(qt * 128, qt * 128 + W)
                            for cc in range(N_CC):
                                cw = CC_W[cc]
                                jsl = slice(cc * 512, cc * 512 + cw)
                                pe = ps.tile([128, 512], f32, tag="big")
                                nc.tensor.matmul(
                                    pe[:W, :cw],
                                    qrT[par : par + 64, hp, qsl],
                                    rT_sb[par : par + 64, hp, jsl],
                                    start=True, stop=True,
                                )
                                if hh == 0:
                                    nc.vector.tensor_copy(
                                        e_sb[:W, qt, jsl], pe[:W, :cw]
                                    )
                                else:
                                    nc.scalar.copy(
                                        e_sb[:W, qt, jsl], pe[:W, :cw]
                                    )
                        h = 2 * hp + hh
                        # write E region (rows qt*128+r, cols [1, T+1))
                        nc.scalar.dma_start(
                            p_epq[h][:, 0:2, 1 : 1 + T], e_sb[:, 0:2, :]
                        )
                        nc.scalar.dma_start(
                            p_epq[h][:16, 2, 1 : 1 + T], e_sb[:16, 2, :]
                        )
                        # corner patches: raw E rows (c+1) cols [0,14) blended
                        for cn in range(2):
                            src0 = 1 if cn == 0 else 245
                            pcs = ps.tile([128, 512], f32, tag="big")
                            nc.tensor.matmul(
                                pcs[:16, :14],
                                qrT[par : par + 64, hp, src0 : src0 + 16],
                                rT_sb[par : par + 64, hp, 0:14],
                                start=True, stop=True,
                            )
                            patch = sm.tile([16, 15], f32, tag="patch")
                            nc.vector.memset(patch[:], 0.0)
                            nc.vector.tensor_copy(patch[:, 1:15], pcs[:16, :14])
                            nc.vector.tensor_tensor(
                                patch[:], patch[:], blend_sb[:, 2 * cn, :], ALU.mult
                            )
                            patchb = sm.tile([16, 15], bf16, tag="patchb")
                            nc.vector.tensor_tensor(
                                patchb[:], patch[:], blend_sb[:, 2 * cn + 1, :], ALU.add
                            )
                            row0 = 0 if cn == 0 else 244
                            tgt = p_h[h][
                                ds(row0 * RW + (T + 1), 16 * RW)
                            ].rearrange("(r c) -> r c", c=RW)[0:16, 0:15]
                            nc.scalar.dma_start(tgt, patchb[:])

                # ---- gathered K/V into SBUF ----
                kT_all = kvp.tile([128, HP, T], bf16, tag="kTall")
                for r in range(N_CORES):
                    src = cag_out[r * C : r * C + C_K].rearrange(
                        "(d p c) -> p d c", d=HP, p=128, c=SPAN
                    )
                    nc.sync.dma_start(
                        kT_all[:, :, r * SPAN : (r + 1) * SPAN], src
                    )
                v_all = kvp.tile([128, 17, D_MODEL], bf16, tag="vall")
                nc.vector.memset(v_all[:, 16, :], 0.0)
                for r in range(N_CORES):
                    g0 = r * SPAN
                    rem = SPAN
                    src_off = r * C + C_K
                    while rem > 0:
                        t_i, p0 = g0 // 128, g0 % 128
                        cnt = min(128 - p0, rem)
                        src = cag_out[
                            src_off : src_off + cnt * D_MODEL
                        ].rearrange("(r d) -> r d", d=D_MODEL)
                        nc.sync.dma_start(v_all[p0 : p0 + cnt, t_i, :], src)
                        g0 += cnt
                        rem -= cnt
                        src_off += cnt * D_MODEL

                # ---- attention pass B ----
                attnT = wts.tile([128, HP, LR], bf16, tag="attnT")
                for hp in range(HP):
                    for hh in range(2):
                        h = 2 * hp + hh
                        par = hh * 64
                        # BD read: rows lr at stride 2T, offset T - pid*SPAN
                        bd_sb = bdp.tile([128, N_QT, T], bf16, tag="bd")
                        off = T - pid * SPAN
                        src = p_h[h][ds(off, N_QT * 128 * 2 * T)].rearrange(
                            "(q p c) -> p q c", q=N_QT, p=128, c=2 * T
                        )[:, :, 0:T]
                        nc.sync.dma_start(bd_sb[:], src)
                        probT = hot.tile([128, 17, LR], bf16, tag="probT")
                        for qt in range(N_QT):
                            W = QT_W[qt]
                            qsl = slice(qt * 128, qt * 128 + W)
                            prob = hot3.tile([128, TPAD], bf16, tag="prob")
                            dens = sm4.tile([128, 8], f32, tag="dens")
                            for cc in range(N_CC):
                                cw = CC_W[cc]
                                jsl = slice(cc * 512, cc * 512 + cw)
                                pa = ps.tile([128, 512], f32, tag="big")
                                nc.tensor.matmul(
                                    pa[:W, :cw],
                                    qwT[par : par + 64, hp, qsl],
                                    kT_all[par : par + 64, hp, jsl],
                                    start=True, stop=False,
                                )
                                nc.tensor.matmul(
                                    pa[:W, :cw],
                                    identb[:W, :W],
                                    bd_sb[:W, qt, jsl],
                                    start=False, stop=True,
                                )
                                nc.scalar.activation(
                                    prob[:W, jsl],
                                    pa[:W, :cw],
                                    AF.Exp, bias=0.0, scale=float(SCALE),
                                    accum_out=dens[:W, cc : cc + 1],
                                )
                            nc.vector.memset(prob[:W, T:], 0.0)
                            den = sm4.tile([128, 2], f32, tag="den")
                            nc.vector.tensor_reduce(
                                den[:W, 0:1], dens[:W, 0:N_CC],
                                axis=mybir.AxisListType.X, op=ALU.add,
                            )
                            rden = sm4.tile([128, 1], f32, tag="rden")
                            nc.vector.reciprocal(rden[:W], den[:W, 0:1])
                            nc.vector.tensor_scalar(
                                out=prob[:W, :], in0=prob[:W, :],
                                scalar1=rden[:W], scalar2=None, op0=ALU.mult,
                            )
                            nc.sync.dma_start_transpose(
                                probT[:, :, qt * 128 : qt * 128 + W], prob[:W, :]
                            )
                        ppv = pspv.tile([64, LR], f32, tag="ppv")
                        for t_i in range(17):
                            nc.tensor.matmul(
                                ppv[:],
                                v_all[:, t_i, h * 64 : h * 64 + 64],
                                probT[:, t_i, :],
                                start=(t_i == 0), stop=(t_i == 16),
                            )
                        nc.scalar.copy(attnT[par : par + 64, hp, :], ppv[:])

                # ---- Wo + residual + LN1 ----
                for qt in range(N_QT):
                    W = QT_W[qt]
                    pw = ps.tile([128, 512], f32, tag="big")
                    for d in range(HP):
                        nc.tensor.matmul(
                            pw[:W, :D_MODEL],
                            attnT[:, d, qt * 128 : qt * 128 + W],
                            wo_sb[:, d, :],
                            start=(d == 0), stop=(d == HP - 1),
                        )
                    x = sm.tile([128, D_MODEL], f32, tag="xres")
                    nc.vector.tensor_tensor(
                        x[:W], w_sb[:W, qt, :], pw[:W, :D_MODEL], ALU.add
                    )
                    _layernorm(
                        nc, sm, w_sb[:W, qt, :], x[:W], W,
                        None if trivial_gb else gb_sb[:W, 0, :],
                        None if trivial_gb else gb_sb[:W, 1, :],
                        eps_sb[:W],
                    )

                # ---- FFN ----
                w1T = wts.tile([128, HP, LR], bf16, tag="w1T")
                for qt in range(N_QT):
                    W = QT_W[qt]
                    for d in range(HP):
                        pt = ps.tile([128, 512], f32, tag="big")
                        nc.tensor.transpose(
                            pt[:, :128], w_sb[:, qt, d * 128 : (d + 1) * 128], ident[:]
                        )
                        nc.scalar.copy(
                            w1T[:, d, qt * 128 : qt * 128 + W], pt[:, :W]
                        )
                pf = [
                    psff.tile([128, 512], f32, tag=f"pf{qt}", name=f"pf{qt}")
                    for qt in range(N_QT)
                ]
                for dc in range(4):
                    w2_sb = w2s.tile([128, HP, D_MODEL], bf16, tag="w2l")
                    nc.sync.dma_start(w2_sb[:], w2c[l, dc])
                    for di4 in range(4):
                        di = dc * 4 + di4
                        phh = ps.tile([128, 512], f32, tag="big")
                        for d in range(HP):
                            nc.tensor.matmul(
                                phh[:, :LR],
                                w1_sb[:, d, di * 128 : (di + 1) * 128],
                                w1T[:, d, :],
                                start=(d == 0), stop=(d == HP - 1),
                            )
                        h1t = sm.tile([128, LR], bf16, tag="h1t")
                        if trivial_b:
                            nc.scalar.activation(
                                h1t[:], phh[:, :LR], AF.Relu, bias=0.0, scale=1.0
                            )
                        else:
                            nc.scalar.activation(
                                h1t[:], phh[:, :LR], AF.Relu,
                                bias=b1_sb[:, di : di + 1], scale=1.0,
                            )
                        for qt in range(N_QT):
                            W = QT_W[qt]
                            nc.tensor.matmul(
                                pf[qt][:W],
                                h1t[:, qt * 128 : qt * 128 + W],
                                w2_sb[:, di4, :],
                                start=(di == 0), stop=(di == 15),
                            )
                for qt in range(N_QT):
                    W = QT_W[qt]
                    x = sm.tile([128, D_MODEL], f32, tag="xres")
                    if trivial_b:
                        nc.vector.tensor_tensor(
                            x[:W], pf[qt][:W], w_sb[:W, qt, :], ALU.add
                        )
                    else:
                        nc.vector.scalar_tensor_tensor(
                            x[:W], pf[qt][:W], 1.0, b2_sb[:W], ALU.mult, ALU.add
                        )
                        nc.vector.tensor_tensor(x[:W], x[:W], w_sb[:W, qt, :], ALU.add)
                    _layernorm(
                        nc, sm, w_sb[:W, qt, :], x[:W], W,
                        None if trivial_gb else gb_sb[:W, 2, :],
                        None if trivial_gb else gb_sb[:W, 3, :],
                        eps_sb[:W],
                    )

            for qt in range(N_QT):
                rows = QT_REAL[qt]
                nc.sync.dma_start(
                    out_t[qt * 128 : qt * 128 + rows, :], w_sb[:rows, qt, :]
                )

    nc.compile()
    return nc


_NC_CACHE = {}
LAST_RESULT = None


def kernel(**inputs):
    trivial_gb = (
        np.all(np.asarray(inputs["ln1_scale"]) == 1.0)
        and np.all(np.asarray(inputs["ln2_scale"]) == 1.0)
        and np.all(np.asarray(inputs["ln1_bias"]) == 0.0)
        and np.all(np.asarray(inputs["ln2_bias"]) == 0.0)
    )
    trivial_b = (
        np.all(np.asarray(inputs["ffn_b1"]) == 0.0)
        and np.all(np.asarray(inputs["ffn_b2"]) == 0.0)
    )
    per_core = _host_prep(inputs)
    drop = []
    if trivial_gb:
        drop.append("gb")
    if trivial_b:
        drop += ["b1col", "b2bc"]
    for pc in per_core:
        for k in drop:
            pc.pop(k, None)
    key = (trivial_gb, trivial_b)
    if key not in _NC_CACHE:
        _NC_CACHE[key] = _build(trivial_gb=trivial_gb, trivial_b=trivial_b)
    res = run_bass_kernel_spmd(
        _NC_CACHE[key], [dict(pc) for pc in per_core], core_ids=list(range(N_CORES))
    )
    global LAST_RESULT
    LAST_RESULT = res
    spans = [res.results[r]["wout"] for r in range(N_CORES)]
    out = np.concatenate(spans, axis=0)
    return np.ascontiguousarray(out[:, None, :].astype(np.float32))


# revision 20
# speedup vs baseline: 1.0590x; 1.0353x over previous
"""Trainium2 Bass kernel for nn_MemTransformerLM (Transformer-XL rel-attention).

Sequence-parallel over 8 NeuronCores: core m owns token span [260m, 260(m+1)).

v2 restructure vs baseline:
- Per-head DRAM rel-shift buffers with row pitch 2T+1: each row is
  [zero][E (T)][mask (T)], where the mask region holds -1e30 so the
  Transformer-XL shift's wrap-read lands on the additive causal mask for
  free (no on-chip mask add). Mem-token corner "garbage" (which the
  reference keeps) is reproduced by patching the first 15 mask cells of the
  32 corner rows each layer with raw E values via tiny matmuls + blends
  driven by per-core blend masks (identity off the corner cores).
- BD is accumulated into the AC PSUM via an identity matmul, and softmax's
  exp reads PSUM directly with the 1/sqrt(d) scale folded into the
  activation, eliminating the separate mask-add/score ops.
- One combined K+V AllGather per layer, overlapped with the (K/V
  independent) E pass.
- Batched 3D-AP DMAs for weights, posT, K/V gather, and E/BD round trips.
"""

import numpy as np
import ml_dtypes

import concourse.bass as bass
import concourse.mybir as mybir
import concourse.tile as tile
from concourse import bacc
from concourse.bass import ds
from concourse.bass_utils import run_bass_kernel_spmd
from concourse.masks import make_identity

BF16 = ml_dtypes.bfloat16
DT = mybir.dt
AF = mybir.ActivationFunctionType
ALU = mybir.AluOpType

N_LAYER = 4
N_HEAD = 8
D_HEAD = 64
D_MODEL = 512
D_INNER = 2048
NMT = 16
T = 2048 + 2 * NMT      # 2080
N_CORES = 8
SPAN = T // N_CORES     # 260
QT_REAL = [128, 128, 4]
QT_W = [128, 128, 16]   # on-chip partition width per q-tile (>=16 for xbar)
N_QT = 3
LR = 272                # local-row extent (2*128 + 16)
NEG = -1e30
SCALE = 1.0 / np.sqrt(D_HEAD)
TPAD = 17 * 128         # 2176
RW = 2 * T + 1          # P-buffer row pitch: [zero][E(T)][M(T)]
P_ROWS = 384
CC_W = [512, 512, 512, 512, 32]  # exp/psum chunking of the j axis
N_CC = 5
HP = 4                  # head-pair partition tiles


def _host_prep(inputs):
    word_emb = np.asarray(inputs["word_emb"], np.float32)
    mem_tokens = np.asarray(inputs["mem_tokens"], np.float32)
    w = np.transpose(word_emb, (1, 0, 2))[:, 0, :]
    mem = mem_tokens[:, 0, :]
    w_full = np.concatenate([mem, w, mem], axis=0)          # [T, 512]

    inv_freq = 1.0 / (10000.0 ** (np.arange(0, D_MODEL, 2, dtype=np.float32) / D_MODEL))
    pos_seq = np.arange(T - 1, -1, -1.0, dtype=np.float32)
    sinusoid = pos_seq[:, None] * inv_freq[None, :]
    pos_emb = np.concatenate([np.sin(sinusoid), np.cos(sinusoid)], axis=-1)
    posT = np.ascontiguousarray(pos_emb.T).astype(BF16)     # [512, T]

    wqkv = np.asarray(inputs["Wqkv"], np.float32).astype(BF16)
    wr = np.asarray(inputs["Wr"], np.float32).astype(BF16)
    wo = np.asarray(inputs["Wo"], np.float32).astype(BF16)
    w1 = np.asarray(inputs["ffn_W1"], np.float32).astype(BF16)
    w2 = np.asarray(inputs["ffn_W2"], np.float32).astype(BF16)
    rwb = np.ascontiguousarray(np.asarray(inputs["r_w_bias"], np.float32).reshape(-1, 1))
    rrb = np.ascontiguousarray(np.asarray(inputs["r_r_bias"], np.float32).reshape(-1, 1))

    ln1g = np.asarray(inputs["ln1_scale"], np.float32)
    ln1b = np.asarray(inputs["ln1_bias"], np.float32)
    ln2g = np.asarray(inputs["ln2_scale"], np.float32)
    ln2b = np.asarray(inputs["ln2_bias"], np.float32)
    gb = np.ascontiguousarray(
        np.stack(
            [
                np.broadcast_to(ln1g[:, None, :], (N_LAYER, 128, D_MODEL)),
                np.broadcast_to(ln1b[:, None, :], (N_LAYER, 128, D_MODEL)),
                np.broadcast_to(ln2g[:, None, :], (N_LAYER, 128, D_MODEL)),
                np.broadcast_to(ln2b[:, None, :], (N_LAYER, 128, D_MODEL)),
            ],
            axis=2,
        ).astype(np.float32)
    )
    b1col = np.ascontiguousarray(
        np.asarray(inputs["ffn_b1"], np.float32).reshape(N_LAYER, D_INNER, 1)
    )
    b2bc = np.ascontiguousarray(
        np.broadcast_to(
            np.asarray(inputs["ffn_b2"], np.float32)[:, None, :],
            (N_LAYER, 128, D_MODEL),
        ).copy()
    )

    # Corner blend masks: tri[c, y] selects the wrap-garbage region
    # (y < 15-c) of the patched mask cells; neg fills the rest with -1e30.
    tri = np.zeros((16, 15), np.float32)
    for c in range(16):
        if c < 15:
            tri[c, : 15 - c] = 1.0
    neg = np.where(tri > 0, 0.0, NEG).astype(np.float32)
    zero16 = np.zeros((16, 15), np.float32)
    negall = np.full((16, 15), NEG, np.float32)

    per_core = []
    for rank in range(N_CORES):
        c0 = rank * SPAN
        w0 = np.zeros((P_ROWS, D_MODEL), np.float32)
        w0[:SPAN] = w_full[c0 : c0 + SPAN]
        blend = np.stack(
            [
                tri if rank == 0 else zero16,
                neg if rank == 0 else negall,
                tri if rank == N_CORES - 1 else zero16,
                neg if rank == N_CORES - 1 else negall,
            ]
        )
        per_core.append(
            {
                "w0": w0,
                "posT": posT,
                "wqkv": wqkv,
                "wr": wr,
                "wo": wo,
                "w1": w1,
                "w2": w2,
                "rwb": rwb,
                "rrb": rrb,
                "blend": np.ascontiguousarray(blend),
                "gb": gb,
                "b1col": b1col,
                "b2bc": b2bc,
            }
        )
    return per_core


def _layernorm(nc, sm, out_ap, x_ap, R, g, b, eps):
    f32 = DT.float32
    stats = sm.tile([128, 6], f32, tag="lnst")
    mv = sm.tile([128, 2], f32, tag="lnmv")
    nc.vector.bn_stats(stats[:R], x_ap)
    nc.vector.bn_aggr(mv[:R], stats[:R])
    std = sm.tile([128, 1], f32, tag="lnstd")
    nc.scalar.activation(std[:R], mv[:R, 1:2], AF.Sqrt, bias=eps, scale=1.0)
    rstd = sm.tile([128, 1], f32, tag="lnrstd")
    nc.vector.reciprocal(rstd[:R], std[:R])
    xn = sm.tile([128, D_MODEL], f32, tag="lnxn")
    if g is None:
        nc.vector.tensor_scalar(
            out=out_ap, in0=x_ap, scalar1=mv[:R, 0:1], scalar2=rstd[:R],
            op0=ALU.subtract, op1=ALU.mult,
        )
    else:
        nc.vector.tensor_scalar(
            out=xn[:R], in0=x_ap, scalar1=mv[:R, 0:1], scalar2=rstd[:R],
            op0=ALU.subtract, op1=ALU.mult,
        )
        nc.vector.tensor_tensor(xn[:R], xn[:R], g, ALU.mult)
        nc.vector.tensor_tensor(out_ap, xn[:R], b, ALU.add)


def _build(trivial_gb=True, trivial_b=True):
    nc = bacc.Bacc("TRN2", num_devices=N_CORES, dynamic_dma_scratch_size=4096)
    f32, bf16 = DT.float32, DT.bfloat16

    w0_t = nc.dram_tensor("w0", [P_ROWS, D_MODEL], f32, kind="ExternalInput")
    posT_t = nc.dram_tensor("posT", [D_MODEL, T], bf16, kind="ExternalInput")
    wqkv_t = nc.dram_tensor("wqkv", [N_LAYER, D_MODEL, 3 * D_MODEL], bf16, kind="ExternalInput")
    wr_t = nc.dram_tensor("wr", [N_LAYER, D_MODEL, D_MODEL], bf16, kind="ExternalInput")
    wo_t = nc.dram_tensor("wo", [N_LAYER, D_MODEL, D_MODEL], bf16, kind="ExternalInput")
    w1_t = nc.dram_tensor("w1", [N_LAYER, D_MODEL, D_INNER], bf16, kind="ExternalInput")
    w2_t = nc.dram_tensor("w2", [N_LAYER, D_INNER, D_MODEL], bf16, kind="ExternalInput")
    rwb_t = nc.dram_tensor("rwb", [D_MODEL, 1], f32, kind="ExternalInput")
    rrb_t = nc.dram_tensor("rrb", [D_MODEL, 1], f32, kind="ExternalInput")
    blend_t = nc.dram_tensor("blend", [4, 16, 15], f32, kind="ExternalInput")
    gb_t = None if trivial_gb else nc.dram_tensor(
        "gb", [N_LAYER, 128, 4, D_MODEL], f32, kind="ExternalInput")
    b1_t = b2_t = None
    if not trivial_b:
        b1_t = nc.dram_tensor("b1col", [N_LAYER, D_INNER, 1], f32, kind="ExternalInput")
        b2_t = nc.dram_tensor("b2bc", [N_LAYER, 128, D_MODEL], f32, kind="ExternalInput")
    out_t = nc.dram_tensor("wout", [SPAN, D_MODEL], f32, kind="ExternalOutput")

    # Per-head rel-shift buffers (+2 margin rows: the BD read AP over-claims).
    p_h = [
        nc.dram_tensor(f"pbuf{h}", [(P_ROWS + 2) * RW], bf16, kind="Internal")
        for h in range(N_HEAD)
    ]
    # views [part, qt, col] with row index qt*128+part at pitch RW
    p_epq = [
        p[0 : P_ROWS * RW].rearrange("(q p c) -> p q c", q=N_QT, p=128, c=RW)
        for p in p_h
    ]

    C_K = D_MODEL * SPAN
    C = 2 * C_K
    cag_in = nc.dram_tensor("cag_in", [C], bf16, kind="Internal")
    cag_out = nc.dram_tensor(
        "cag_out", [N_CORES * C], bf16, kind="Internal", addr_space="Shared"
    )
    cag_in_k = cag_in[0:C_K].rearrange("(d p c) -> p d c", d=HP, p=128, c=SPAN)
    rg = [list(range(N_CORES))]

    # batched weight views
    wqkv4 = wqkv_t.rearrange("l (dp p) n -> l p dp n", p=128)
    wr4 = wr_t.rearrange("l (dp p) n -> l p dp n", p=128)
    wo4 = wo_t.rearrange("l (dp p) n -> l p dp n", p=128)
    w14 = w1_t.rearrange("l (dp p) n -> l p dp n", p=128)
    w2c = w2_t.rearrange("l (c4 dp p) n -> l c4 p dp n", c4=4, p=128)
    posT4 = posT_t.rearrange("(dp p) c -> p dp c", p=128)

    with tile.TileContext(nc, num_cores=N_CORES) as tc:
        pid = nc.sync.partition_id()
        with (
            tc.tile_pool(name="const", bufs=1) as constp,
            tc.tile_pool(name="pers", bufs=1) as pers,
            tc.tile_pool(name="wts", bufs=1) as wts,
            tc.tile_pool(name="w2s", bufs=2) as w2s,
            tc.tile_pool(name="kv", bufs=1) as kvp,
            tc.tile_pool(name="eh", bufs=2) as ehp,
            tc.tile_pool(name="bdp", bufs=2) as bdp,
            tc.tile_pool(name="hot", bufs=2) as hot,
            tc.tile_pool(name="hot3", bufs=3) as hot3,
            tc.tile_pool(name="sm", bufs=2) as sm,
            tc.tile_pool(name="sm4", bufs=4) as sm4,
            tc.tile_pool(name="ps", bufs=4, space="PSUM") as ps,
            tc.tile_pool(name="pspv", bufs=1, space="PSUM") as pspv,
            tc.tile_pool(name="psff", bufs=1, space="PSUM") as psff,
        ):
            ident = constp.tile([128, 128], f32)
            make_identity(nc, ident[:])
            identb = constp.tile([128, 128], bf16)
            nc.vector.tensor_copy(identb[:], ident[:])
            blend_sb = constp.tile([16, 4, 15], f32)
            nc.sync.dma_start(
                blend_sb[:], blend_t.rearrange("a p c -> p a c")
            )
            eps_sb = constp.tile([128, 1], f32)
            nc.vector.memset(eps_sb[:], 1e-5)
            rwb_sb = constp.tile([128, HP], f32)
            rrb_sb = constp.tile([128, HP], f32)
            for d in range(HP):
                nc.sync.dma_start(rwb_sb[:, d : d + 1], rwb_t[d * 128 : (d + 1) * 128, :])
                nc.sync.dma_start(rrb_sb[:, d : d + 1], rrb_t[d * 128 : (d + 1) * 128, :])

            # init P buffers to -1e30 everywhere (mask region + pad rows);
            # issued on the scalar HWDGE ring so layer-0 weight loads on the
            # sync ring are not queued behind 25 MB of fill.
            negsmall = sm.tile([128, 2112], bf16, tag="scr4k")
            nc.vector.memset(negsmall[:], NEG)
            for h in range(N_HEAD):
                for q in range(N_QT):
                    nc.scalar.dma_start(p_epq[h][:, q, 0:2112], negsmall[:])
                    nc.scalar.dma_start(
                        p_epq[h][:, q, 2112:RW], negsmall[:, 0 : RW - 2112]
                    )
                tail = p_h[h][ds(P_ROWS * RW, 2 * RW)].rearrange(
                    "(r c) -> r c", c=RW
                )
                nc.scalar.dma_start(tail[:, 0:2112], negsmall[:2, :])
                nc.scalar.dma_start(tail[:, 2112:RW], negsmall[:2, 0 : RW - 2112])

            w_sb = pers.tile([128, N_QT, D_MODEL], f32)
            nc.sync.dma_start(
                w_sb[:],
                w0_t.rearrange("(q p) d -> p q d", p=128),
            )

            for l in range(N_LAYER):
                # ---- layer weights ----
                wqkv_sb = wts.tile([128, HP, 3 * D_MODEL], bf16, tag="wqkv")
                wr_sb = wts.tile([128, HP, D_MODEL], bf16, tag="wrl")
                wo_sb = wts.tile([128, HP, D_MODEL], bf16, tag="wol")
                w1_sb = wts.tile([128, HP, D_INNER], bf16, tag="w1l")
                nc.sync.dma_start(wqkv_sb[:], wqkv4[l])
                nc.sync.dma_start(wr_sb[:], wr4[l])
                nc.sync.dma_start(wo_sb[:], wo4[l])
                nc.sync.dma_start(w1_sb[:], w14[l])
                gb_sb = None
                if not trivial_gb:
                    gb_sb = wts.tile([128, 4, D_MODEL], f32, tag="gbl")
                    nc.sync.dma_start(gb_sb[:], gb_t[l])
                b1_sb = None
                b2_sb = None
                if not trivial_b:
                    b1_sb = wts.tile([128, 16], f32, tag="b1l")
                    for d in range(16):
                        nc.sync.dma_start(b1_sb[:, d : d + 1], b1_t[l, d * 128 : (d + 1) * 128, :])
                    b2_sb = wts.tile([128, D_MODEL], f32, tag="b2l")
                    nc.sync.dma_start(b2_sb[:], b2_t[l])

                # ---- transpose residual -> wT bf16 [128, hp, LR] ----
                wT_sb = wts.tile([128, HP, LR], bf16, tag="wT")
                for qt in range(N_QT):
                    W = QT_W[qt]
                    for d in range(HP):
                        pt = ps.tile([128, 512], f32, tag="big")
                        nc.tensor.transpose(
                            pt[:, :128], w_sb[:, qt, d * 128 : (d + 1) * 128], ident[:]
                        )
                        nc.scalar.copy(
                            wT_sb[:, d, qt * 128 : qt * 128 + W], pt[:, :W]
                        )

                # ---- projections ----
                qwT = wts.tile([128, HP, LR], bf16, tag="qwT")
                qrT = wts.tile([128, HP, LR], bf16, tag="qrT")
                kT_own = wts.tile([128, HP, SPAN], bf16, tag="kTown")
                for hp in range(HP):
                    pq = ps.tile([128, 512], f32, tag="big")
                    for d in range(HP):
                        nc.tensor.matmul(
                            pq[:, :LR],
                            wqkv_sb[:, d, hp * 128 : hp * 128 + 128],
                            wT_sb[:, d, :],
                            start=(d == 0), stop=(d == HP - 1),
                        )
                    nc.scalar.activation(
                        qwT[:, hp, :], pq[:, :LR], AF.Identity,
                        bias=rwb_sb[:, hp : hp + 1], scale=1.0,
                    )
                    nc.scalar.activation(
                        qrT[:, hp, :], pq[:, :LR], AF.Identity,
                        bias=rrb_sb[:, hp : hp + 1], scale=1.0,
                    )
                    pk = ps.tile([128, 512], f32, tag="big")
                    for d in range(HP):
                        nc.tensor.matmul(
                            pk[:, :LR],
                            wqkv_sb[:, d, D_MODEL + hp * 128 : D_MODEL + hp * 128 + 128],
                            wT_sb[:, d, :],
                            start=(d == 0), stop=(d == HP - 1),
                        )
                    nc.scalar.copy(kT_own[:, hp, :], pk[:, :SPAN])
                nc.sync.dma_start(cag_in_k, kT_own[:])

                v_own = sm.tile([128, N_QT, D_MODEL], bf16, tag="scr4k")
                for qt in range(N_QT):
                    W = QT_W[qt]
                    pv = ps.tile([128, 512], f32, tag="big")
                    for d in range(HP):
                        nc.tensor.matmul(
                            pv[:W, :D_MODEL],
                            wT_sb[:, d, qt * 128 : qt * 128 + W],
                            wqkv_sb[:, d, 2 * D_MODEL :],
                            start=(d == 0), stop=(d == HP - 1),
                        )
                    nc.vector.tensor_copy(v_own[:W, qt, :], pv[:W, :D_MODEL])
                    rows = QT_REAL[qt]
                    vtgt = cag_in[
                        C_K + (qt * 128) * D_MODEL : C_K + (qt * 128 + rows) * D_MODEL
                    ].rearrange("(r d) -> r d", d=D_MODEL)
                    nc.sync.dma_start(vtgt, v_own[:rows, qt, :])

                nc.gpsimd.collective_compute(
                    "AllGather", ALU.bypass, replica_groups=rg,
                    ins=[cag_in[:]], outs=[cag_out[:]],
                )

                # ---- r^T (posT streamed from DRAM), K/V-independent ----
                rT_sb = wts.tile([128, HP, T], bf16, tag="rT")
                for ch in range(5):
                    cw = min(512, T - ch * 512)
                    pos_big = sm.tile([128, HP, 512], bf16, tag="scr4k")
                    nc.sync.dma_start(
                        pos_big[:, :, :cw], posT4[:, :, ch * 512 : ch * 512 + cw]
                    )
                    for hp in range(HP):
                        pr = ps.tile([128, 512], f32, tag="big")
                        for d in range(HP):
                            nc.tensor.matmul(
                                pr[:, :cw],
                                wr_sb[:, d, hp * 128 : hp * 128 + 128],
                                pos_big[:, d, :cw],
                                start=(d == 0), stop=(d == HP - 1),
                            )
                        nc.scalar.copy(
                            rT_sb[:, hp, ch * 512 : ch * 512 + cw], pr[:, :cw]
                        )

                # ---- E pass (writes rel-shift buffers; overlaps AllGather) ----
                for hp in range(HP):
                    for hh in range(2):
                        par = hh * 64
                        e_sb = ehp.tile([128, N_QT, T], bf16, tag="eh")
                        for qt in range(N_QT):
                            W = QT_W[qt]
                            qsl = slice(qt * 128, qt * 128 + W)
                            for cc in range(N_CC):
                                cw = CC_W[cc]
                                jsl = slice(cc * 512, cc * 512 + cw)
                                pe = ps.tile([128, 512], f32, tag="big")
                                nc.tensor.matmul(
                                    pe[:W, :cw],
                                    qrT[par : par + 64, hp, qsl],
                                    rT_sb[par : par + 64, hp, jsl],
                                    start=True, stop=True,
                                )
                                nc.vector.tensor_copy(
                                    e_sb[:W, qt, jsl], pe[:W, :cw]
                                )
                        h = 2 * hp + hh
                        # write E region (rows qt*128+r, cols [1, T+1))
                        nc.sync.dma_start(
                            p_epq[h][:, 0:2, 1 : 1 + T], e_sb[:, 0:2, :]
                        )
                        nc.sync.dma_start(
                            p_epq[h][:16, 2, 1 : 1 + T], e_sb[:16, 2, :]
                        )
                        # corner patches: raw E rows (c+1) cols [0,14) blended
                        for cn in range(2):
                            src0 = 1 if cn == 0 else 245
                            pcs = ps.tile([128, 512], f32, tag="big")
                            nc.tensor.matmul(
                                pcs[:16, :14],
                                qrT[par : par + 64, hp, src0 : src0 + 16],
                                rT_sb[par : par + 64, hp, 0:14],
                                start=True, stop=True,
                            )
                            patch = sm.tile([16, 15], f32, tag="patch")
                            nc.vector.memset(patch[:], 0.0)
                            nc.vector.tensor_copy(patch[:, 1:15], pcs[:16, :14])
                            nc.vector.tensor_tensor(
                                patch[:], patch[:], blend_sb[:, 2 * cn, :], ALU.mult
                            )
                            patchb = sm.tile([16, 15], bf16, tag="patchb")
                            nc.vector.tensor_tensor(
                                patchb[:], patch[:], blend_sb[:, 2 * cn + 1, :], ALU.add
                            )
                            row0 = 0 if cn == 0 else 244
                            tgt = p_h[h][
                                ds(row0 * RW + (T + 1), 16 * RW)
                            ].rearrange("(r c) -> r c", c=RW)[0:16, 0:15]
                            nc.sync.dma_start(tgt, patchb[:])

                # ---- gathered K/V into SBUF ----
                kT_all = kvp.tile([128, HP, T], bf16, tag="kTall")
                for r in range(N_CORES):
                    src = cag_out[r * C : r * C + C_K].rearrange(
                        "(d p c) -> p d c", d=HP, p=128, c=SPAN
                    )
                    nc.sync.dma_start(
                        kT_all[:, :, r * SPAN : (r + 1) * SPAN], src
                    )
                v_all = kvp.tile([128, 17, D_MODEL], bf16, tag="vall")
                nc.vector.memset(v_all[:, 16, :], 0.0)
                for r in range(N_CORES):
                    g0 = r * SPAN
                    rem = SPAN
                    src_off = r * C + C_K
                    while rem > 0:
                        t_i, p0 = g0 // 128, g0 % 128
                        cnt = min(128 - p0, rem)
                        src = cag_out[
                            src_off : src_off + cnt * D_MODEL
                        ].rearrange("(r d) -> r d", d=D_MODEL)
                        nc.sync.dma_start(v_all[p0 : p0 + cnt, t_i, :], src)
                        g0 += cnt
                        rem -= cnt
                        src_off += cnt * D_MODEL

                # ---- attention pass B ----
                attnT = wts.tile([128, HP, LR], bf16, tag="attnT")
                for hp in range(HP):
                    for hh in range(2):
                        h = 2 * hp + hh
                        par = hh * 64
                        # BD read: rows lr at stride 2T, offset T - pid*SPAN
                        bd_sb = bdp.tile([128, N_QT, T], bf16, tag="bd")
                        off = T - pid * SPAN
                        src = p_h[h][ds(off, N_QT * 128 * 2 * T)].rearrange(
                            "(q p c) -> p q c", q=N_QT, p=128, c=2 * T
                        )[:, :, 0:T]
                        nc.sync.dma_start(bd_sb[:], src)
                        probT = hot.tile([128, 17, LR], bf16, tag="probT")
                        for qt in range(N_QT):
                            W = QT_W[qt]
                            qsl = slice(qt * 128, qt * 128 + W)
                            prob = hot3.tile([128, TPAD], bf16, tag="prob")
                            dens = sm4.tile([128, 8], f32, tag="dens")
                            for cc in range(N_CC):
                                cw = CC_W[cc]
                                jsl = slice(cc * 512, cc * 512 + cw)
                                pa = ps.tile([128, 512], f32, tag="big")
                                nc.tensor.matmul(
                                    pa[:W, :cw],
                                    qwT[par : par + 64, hp, qsl],
                                    kT_all[par : par + 64, hp, jsl],
                                    start=True, stop=False,
                                )
                                nc.tensor.matmul(
                                    pa[:W, :cw],
                                    identb[:W, :W],
                                    bd_sb[:W, qt, jsl],
                                    start=False, stop=True,
                                )
                                nc.scalar.activation(
                                    prob[:W, jsl],
                                    pa[:W, :cw],
                                    AF.Exp, bias=0.0, scale=float(SCALE),
                                    accum_out=dens[:W, cc : cc + 1],
                                )
                            nc.vector.memset(prob[:W, T:], 0.0)
                            den = sm4.tile([128, 2], f32, tag="den")
                            nc.vector.tensor_reduce(
                                den[:W, 0:1], dens[:W, 0:N_CC],
                                axis=mybir.AxisListType.X, op=ALU.add,
                            )
                            rden = sm4.tile([128, 1], f32, tag="rden")
                            nc.vector.reciprocal(rden[:W], den[:W, 0:1])
                            nc.vector.tensor_scalar(
                                out=prob[:W, :], in0=prob[:W, :],
                                scalar1=rden[:W], scalar2=None, op0=ALU.mult,
                            )
                            nc.sync.dma_start_transpose(
                                probT[:, :, qt * 128 : qt * 128 + W], prob[:W, :]
                            )
                        ppv = pspv.tile([64, LR], f32, tag="ppv")
                        for t_i in range(17):
                            nc.tensor.matmul(
                                ppv[:],
                                v_all[:, t_i, h * 64 : h * 64 + 64],
                                probT[:, t_i, :],
                                start=(t_i == 0), stop=(t_i == 16),
                            )
                        nc.scalar.copy(attnT[par : par + 64, hp, :], ppv[:])

                # ---- Wo + residual + LN1 ----
                for qt in range(N_QT):
                    W = QT_W[qt]
                    pw = ps.tile([128, 512], f32, tag="big")
                    for d in range(HP):
                        nc.tensor.matmul(
                            pw[:W, :D_MODEL],
                            attnT[:, d, qt * 128 : qt * 128 + W],
                            wo_sb[:, d, :],
                            start=(d == 0), stop=(d == HP - 1),
                        )
                    x = sm.tile([128, D_MODEL], f32, tag="xres")
                    nc.vector.tensor_tensor(
                        x[:W], w_sb[:W, qt, :], pw[:W, :D_MODEL], ALU.add
                    )
                    _layernorm(
                        nc, sm, w_sb[:W, qt, :], x[:W], W,
                        None if trivial_gb else gb_sb[:W, 0, :],
                        None if trivial_gb else gb_sb[:W, 1, :],
                        eps_sb[:W],
                    )

                # ---- FFN ----
                w1T = wts.tile([128, HP, LR], bf16, tag="w1T")
                for qt in range(N_QT):
                    W = QT_W[qt]
                    for d in range(HP):
                        pt = ps.tile([128, 512], f32, tag="big")
                        nc.tensor.transpose(
                            pt[:, :128], w_sb[:, qt, d * 128 : (d + 1) * 128], ident[:]
                        )
                        nc.scalar.copy(
                            w1T[:, d, qt * 128 : qt * 128 + W], pt[:, :W]
                        )
                pf = [
                    psff.tile([128, 512], f32, tag=f"pf{qt}", name=f"pf{qt}")
                    for qt in range(N_QT)
                ]
                for dc in range(4):
                    w2_sb = w2s.tile([128, HP, D_MODEL], bf16, tag="w2l")
                    nc.sync.dma_start(w2_sb[:], w2c[l, dc])
                    for di4 in range(4):
                        di = dc * 4 + di4
                        phh = ps.tile([128, 512], f32, tag="big")
                        for d in range(HP):
                            nc.tensor.matmul(
                                phh[:, :LR],
                                w1_sb[:, d, di * 128 : (di + 1) * 128],
                                w1T[:, d, :],
                                start=(d == 0), stop=(d == HP - 1),
                            )
                        h1t = sm.tile([128, LR], bf16, tag="h1t")
                        if trivial_b:
                            nc.scalar.activation(
                                h1t[:], phh[:, :LR], AF.Relu, bias=0.0, scale=1.0
                            )
                        else:
                            nc.scalar.activation(
                                h1t[:], phh[:, :LR], AF.Relu,
                                bias=b1_sb[:, di : di + 1], scale=1.0,
                            )
                        for qt in range(N_QT):
                            W = QT_W[qt]
                            nc.tensor.matmul(
                                pf[qt][:W],
                                h1t[:, qt * 128 : qt * 128 + W],
                                w2_sb[:, di4, :],
                                start=(di == 0), stop=(di == 15),
                            )
                for qt in range(N_QT):
                    W = QT_W[qt]
                    x = sm.tile([128, D_MODEL], f32, tag="xres")
                    if trivial_b:
                        nc.vector.tensor_tensor(
                            x[:W], pf[qt][:W], w_sb[:W, qt, :], ALU.add
                        )
                    else:
                        nc.vector.scalar_tensor_tensor(
                            x[:W], pf[qt][:W], 1.0, b2_sb[:W], ALU.mult, ALU.add
                        )
                        nc.vector.tensor_tensor(x[:W], x[:W], w_sb[:W, qt, :], ALU.add)
                    _layernorm(
                        nc, sm, w_sb[:W, qt, :], x[:W], W,
                        None if trivial_gb else gb_sb[:W, 2, :],
                        None if trivial_gb else gb_sb[:W, 3, :],
                        eps_sb[:W],
                    )

            for qt in range(N_QT):
                rows = QT_REAL[qt]
                nc.sync.dma_start(
                    out_t[qt * 128 : qt * 128 + rows, :], w_sb[:rows, qt, :]
                )

    nc.compile()
    return nc


_NC_CACHE = {}
LAST_RESULT = None


def kernel(**inputs):
    trivial_gb = (
        np.all(np.asarray(inputs["ln1_scale"]) == 1.0)
        and np.all(np.asarray(inputs["ln2_scale"]) == 1.0)
        and np.all(np.asarray(inputs["ln1_bias"]) == 0.0)
        and np.all(np.asarray(inputs["ln2_bias"]) == 0.0)
    )
    trivial_b = (
        np.all(np.asarray(inputs["ffn_b1"]) == 0.0)
        and np.all(np.asarray(inputs["ffn_b2"]) == 0.0)
    )
    per_core = _host_prep(inputs)
    drop = []
    if trivial_gb:
        drop.append("gb")
    if trivial_b:
        drop += ["b1col", "b2bc"]
    for pc in per_core:
        for k in drop:
            pc.pop(k, None)
    key = (trivial_gb, trivial_b)
    if key not in _NC_CACHE:
        _NC_CACHE[key] = _build(trivial_gb=trivial_gb, trivial_b=trivial_b)
    res = run_bass_kernel_spmd(
        _NC_CACHE[key], [dict(pc) for pc in per_core], core_ids=list(range(N_CORES))
    )
    global LAST_RESULT
    LAST_RESULT = res
    spans = [res.results[r]["wout"] for r in range(N_CORES)]
    out = np.concatenate(spans, axis=0)
    return np.ascontiguousarray(out[:, None, :].astype(np.float32))
